# revision 19
# baseline (speedup 1.0000x reference)
"""Cross-attention (1x1-conv q/k/v + softmax(Q^T K) + V@attn^T) on Trainium2.

Data-parallel over batch: 8 batches -> 8 NeuronCores, one full [N,N]
attention per core; the small CxC projection weights are replicated.

Per-core device program (all matmuls, zero transposes). The two score
projections are folded into one on the host: scores = (Wq x1)^T (Wk x2)
= x1^T G x2 with G = Wk^T Wq [CxC], so x1 feeds the score matmuls raw:
  A[c,m]   = G.T @ x2              (fp16 matmul, c on partitions)
  vT[m,c'] = x2.T @ WvT            (fp16 matmul, bf16 result; appended
                                    ones column c'=C)
  sT[m,n]  = A.T @ x1              (fp16 scores, transposed layout)
  pT[m,n]  = exp(sT - SHIFT)       (ScalarE, bf16 out; SHIFT makes per-row max
                                    subtraction unnecessary: softmax is
                                    shift-invariant and scores stay in
                                    [-150, ~110] => exp in fp32/bf16 range)
  o'[n,c'] = pT.T @ vT             (bf16; ones column accumulates row sums)
  outT[n,c] = o'[n,:C] * (1/o'[n,C])

dtype choices: the wire + score path is fp16 (e5m10). x inputs are ~N(0,1)
so fp16's range is ample, and its 10-bit mantissa matches what the PE keeps
internally for fp32r (tf32-ish) operands - measured end-to-end error is the
same as the fp32r build (~4e-3 absmax-relative) while the DMA bytes halve
and, crucially, fp16 weights get FWL + a separate LDWEIGHTS that overlaps
the previous matmul: fp32r matmuls self-load their stationary operand and
pay ~+15ns each (227-235ns vs the 216ns FD=512 roofline; fp16 reaches it).
The value path (pT, vT) is bf16 because pT = exp(s-SHIFT) spans e^-200..e^50
which overflows fp16's e^11 range. Output is written fp16 (|out| <= ~6).

The host reassembles outT -> [B, C, H, W].

Biases are not applied: the problem spec fixes bq/bk/bv to zeros.
"""

from contextlib import ExitStack

import numpy as np

import concourse.bass as bass
import concourse.mybir as mybir
import concourse.tile as tile
from concourse import bacc, bass_utils

B, C, H, W = 8, 256, 64, 64
N = H * W          # 4096 tokens per image
P = 128            # partition count
KC = C // P        # 2 contraction chunks over channels
NMM = N // P       # 32 key-side chunks
SB = 512           # query-side superblock (score matmul free dim)
NSB = N // SB      # 8
C2 = C + 1         # value width + ones column (bf16 matmuls allow odd free)
SHIFT = 60.0       # softmax exp shift (see module docstring)
NWARM = 20         # FD=128 warmup matmuls (~2.1us cold) to start the HAM
                   # clock-gate activity window while the input DMA flies

# x chunk column ranges; host packs each chunk contiguously per partition
# (one DMA descriptor per partition per chunk instead of one per
# (partition, kc) pair - descriptor count, not bytes, limits the head).
X2_RANGES = [(0, 256), (256, 512), (512, 1024), (1024, 2048), (2048, 3072),
             (3072, 4096)]
X1_RANGES = [(0, 512), (512, 4096)]

_CACHE: dict = {}
TRACE = False       # set by test harness to capture an NTFF profile
TRACE_DIR = None    # optional fixed profile output dir


def _build_program():
    f32 = mybir.dt.float32
    f16 = mybir.dt.float16    # wire + score path
    bf16 = mybir.dt.bfloat16  # value path: range for exp(s-SHIFT)
    exp = mybir.ActivationFunctionType.Exp
    # bacc (not raw Bass): its compile() pass splits multi-semaphore waits,
    # which walrus codegen requires (one wait per TPB instruction).
    nc = bacc.Bacc("TRN2", target_bir_lowering=False, debug=False)

    # x/w arrive pre-packed: partition-major, each chunk's two kc halves
    # adjacent, so every chunk DMA is one contiguous descriptor/partition.
    x1_d = nc.dram_tensor("x1", [P, KC * N], f16, kind="ExternalInput").ap()
    x2_d = nc.dram_tensor("x2", [P, KC * N], f16, kind="ExternalInput").ap()
    wk_d = nc.dram_tensor("wkT", [P, KC * C], f16, kind="ExternalInput").ap()
    wv_d = nc.dram_tensor("wvT", [P, KC * C], f16, kind="ExternalInput").ap()
    outT_d = nc.dram_tensor("outT", [N, C], f16, kind="ExternalOutput").ap()

    with tile.TileContext(nc) as tc:
        with ExitStack() as ctx:
            consts = ctx.enter_context(tc.tile_pool(name="consts", bufs=1))
            acts = ctx.enter_context(tc.tile_pool(name="acts", bufs=1))

            # PE warmup source: memset early on the (otherwise idle) GpSimd
            # so the dummy matmuls only wait on it, not on any DMA.
            dummy = consts.tile([P, SB], f16, name="dummy")
            nc.gpsimd.memset(dummy, 0.0)

            nbias = consts.tile([P, 1], f32)
            nc.vector.memset(nbias, -SHIFT)

            # x tiles. Quarter granularity, except the first two transfers
            # are 512-col eighths so the first kq-projection + scores can
            # start as early as possible.  x1 beyond the first superblock
            # isn't needed until the steady-state loop, so it ships as one
            # big transfer at the back of the chain.
            xpool = ctx.enter_context(tc.tile_pool(name="xpool", bufs=1))
            x2_sb = [xpool.tile([P, KC, b - a], f16, name=f"x2_{a}")
                     for a, b in X2_RANGES]
            x1_sb = [xpool.tile([P, KC, b - a], f16, name=f"x1_{a}")
                     for a, b in X1_RANGES]

            def xslice(tiles, ranges, c0, c1):
                for t, (a, b) in zip(tiles, ranges):
                    if a <= c0 and c1 <= b:
                        return t[:, :, c0 - a:c1 - a]
                raise AssertionError((c0, c1))

            # DMA schedule.  The SDMA engines round-robin across all queued
            # transfers, so with no ordering everything finishes together
            # and the PE starves; but each completion->trigger handoff costs
            # ~2us of doorbell->data latency, so a fully serial chain wastes
            # ~2us per link.  Compromise: 4 priority GROUPS - members of a
            # group shard across the 16 queues concurrently, the next group
            # waits on the previous one.  Group 1 is exactly what the first
            # k-projection + first scores need.  Triggers ride on the Scalar
            # sequencer - its "main" starts ~1.2us before Sync's.
            w_sb = {nm: consts.tile([P, KC, C], f16, name=f"{nm}_sb")
                    for nm in ("wk", "wv")}

            def xsrc(src, a, b):
                return src[:, KC * a:KC * b].rearrange(
                    "p (kc w) -> p kc w", kc=KC)

            # Each group's trigger waits on an EARLY member of the previous
            # group (not full completion): the ~2.7us doorbell->data latency
            # then overlaps the previous group's in-flight tail instead of
            # stacking after it.  Triggers execute in order on the scalar
            # queue, so ordering stays monotone.
            transfers = [
                ("wk", w_sb["wk"], xsrc(wk_d, 0, C), None),
                ("x2s0", x2_sb[0], xsrc(x2_d, 0, 256), None),
                ("x2s1", x2_sb[1], xsrc(x2_d, 256, 512), "wk"),
                ("x1e0", x1_sb[0], xsrc(x1_d, 0, 512), "wk"),
                ("wv", w_sb["wv"], xsrc(wv_d, 0, C), "x2s0"),
                ("x2e1", x2_sb[2], xsrc(x2_d, 512, 1024), "x2s0"),
                ("x2q1", x2_sb[3], xsrc(x2_d, 1024, 2048), "x2s1"),
                ("x2q2", x2_sb[4], xsrc(x2_d, 2048, 3072), "x2e1"),
                ("x2q3", x2_sb[5], xsrc(x2_d, 3072, 4096), "x2e1"),
                ("x1rest", x1_sb[1], xsrc(x1_d, 512, 4096), "x2e1"),
            ]
            dmas = {}
            for nm, dst, src, dep in transfers:
                dma = nc.scalar.dma_start(out=dst, in_=src)
                if dep is not None:
                    tile.add_dep_helper(dma.ins, dmas[dep].ins,
                                        reason="dma priority group")
                dmas[nm] = dma

            # q/k as per-superblock tiles, vT per m-chunk: fine-grained deps
            # let scores/out matmuls start before all projections finish.
            k_sb = [acts.tile([P, KC, SB], f16, name=f"k_{ns}", bufs=1)
                    for ns in range(NSB)]
            vT_sb = [acts.tile([P, C2], bf16, name=f"vT_{mm}", bufs=1)
                     for mm in range(NMM)]
            for mm in range(NMM):
                nc.vector.memset(vT_sb[mm][:, C:C2], 1.0)

            # ---- pools (ps/po PSUM rotations are shared by projections
            # and the attention loop; 6 + 2 = all 8 banks) ----
            pts = ctx.enter_context(tc.tile_pool(name="pts", bufs=24))
            ps_pool = ctx.enter_context(tc.tile_pool(name="ps", bufs=3, space="PSUM"))
            po_pool = ctx.enter_context(tc.tile_pool(name="po", bufs=2, space="PSUM"))
            outp = ctx.enter_context(tc.tile_pool(name="outp", bufs=4))
            normp = ctx.enter_context(tc.tile_pool(name="normp", bufs=4))

            # ---- PE warmup: the HAM clock gate holds the PE at 1.2 GHz
            # until ~3.4us of sustained activity.  Burn that window on
            # dummy matmuls while the x DMAs are in flight so the real
            # prologue runs at 2.4 GHz.  FD=128 keeps the drain short if
            # the first data lands mid-burst.
            for wmm in range(0, NWARM, 8):
                pw = ps_pool.tile([P, 2, SB], f32, tag="ps", name=f"warm{wmm}")
                for i in range(min(8, NWARM - wmm)):
                    nc.tensor.matmul(
                        pw[:, i % 2, (i // 2) * P:(i // 2 + 1) * P],
                        lhsT=dummy[:, 0:P], rhs=dummy[:, 0:P],
                        start=True, stop=True)

            def emit_kqproj(ns, lo=0, hi=SB):
                # k_sb[ns][:, :, lo:hi] from x2 cols [ns*SB+lo, ns*SB+hi);
                # kc-outer so consecutive matmuls alternate PSUM banks
                xs = xslice(x2_sb, X2_RANGES, ns * SB + lo, ns * SB + hi)
                pq = ps_pool.tile([P, 2, hi - lo], f32, tag="ps",
                                  name=f"pq_{ns}_{lo}")
                # full-width tiles span 2 PSUM banks: kc-outer alternates
                # banks.  half-width tiles fit ONE bank: mo-outer, so the
                # first accumulation group closes before the second opens.
                loop = ([(kc, mo) for kc in range(KC) for mo in range(KC)]
                        if hi - lo == SB else
                        [(kc, mo) for mo in range(KC) for kc in range(KC)])
                for kc, mo in loop:
                    nc.tensor.matmul(
                        pq[:, mo, :],
                        lhsT=w_sb["wk"][:, kc, mo * P:(mo + 1) * P],
                        rhs=xs[:, kc, :],
                        start=(kc == 0), stop=(kc == KC - 1))
                for mo in range(KC):
                    nc.vector.tensor_copy(out=k_sb[ns][:, mo, lo:hi],
                                          in_=pq[:, mo, :])

            def emit_vproj(mm0, count):
                # m-chunks [mm0, mm0+count) of the value projection; pairs
                # of accumulators from the po rotation alternate banks
                for pr in range(count // 2):
                    pv = [po_pool.tile([P, C], f32, tag="po",
                                       name=f"pv_{mm0}_{pr}_{i}")
                          for i in range(2)]
                    for kc in range(KC):
                        for i in range(2):
                            mm = mm0 + pr * 2 + i
                            xs = xslice(x2_sb, X2_RANGES, mm * P, (mm + 1) * P)
                            nc.tensor.matmul(
                                pv[i],
                                lhsT=xs[:, kc, :],
                                rhs=w_sb["wv"][:, kc, :],
                                start=(kc == 0), stop=(kc == KC - 1))
                    for i in range(2):
                        nc.vector.tensor_copy(
                            out=vT_sb[mm0 + pr * 2 + i][:, 0:C],
                            in_=pv[i])

            def emit_scores(sb, t, pt_tiles):
                xq = xslice(x1_sb, X1_RANGES, sb * SB, (sb + 1) * SB)
                ps = ps_pool.tile([P, 2, SB], f32, tag="ps",
                                  name=f"ps_{sb}_{t}")
                for kc in range(KC):   # kc-outer: banks alternate A B A B
                    for i in range(2):
                        koff = (t * 2 + i) * P
                        kt = k_sb[koff // SB]
                        nc.tensor.matmul(
                            ps[:, i, :],
                            lhsT=kt[:, kc, koff % SB:koff % SB + P],
                            rhs=xq[:, kc, :],
                            start=(kc == 0), stop=(kc == KC - 1))
                pt = pts.tile([P, 2, SB], bf16, tag="pt")
                nc.scalar.activation(out=pt, in_=ps, func=exp,
                                     bias=nbias, scale=1.0)
                pt_tiles.append(pt)

            def emit_out(sb, pt_tiles):
                # j-outer: one live out-accumulator bank at a time.
                for j in range(SB // P):
                    po = po_pool.tile([P, C2], f32, tag="po",
                                      name=f"po_{sb}_{j}")
                    for mm in range(NMM):
                        nc.tensor.matmul(
                            po,
                            lhsT=pt_tiles[mm // 2][:, mm % 2,
                                                   j * P:(j + 1) * P],
                            rhs=vT_sb[mm],
                            start=(mm == 0), stop=(mm == NMM - 1))
                    rc = normp.tile([P, 1], f32, tag="rc")
                    nc.vector.reciprocal(rc, po[:, C:C + 1])
                    ot = outp.tile([P, C], f16, tag="ot")
                    nc.vector.tensor_scalar_mul(ot, po[:, 0:C], rc)
                    n0 = sb * SB + j * P
                    # two half-height DMAs land on two queues -> the
                    # epilogue's final transfer drains ~2x faster.
                    nc.sync.dma_start(out=outT_d[n0:n0 + P // 2, :],
                                      in_=ot[0:P // 2])
                    nc.sync.dma_start(out=outT_d[n0 + P // 2:n0 + P, :],
                                      in_=ot[P // 2:P])

            # ---- prologue: k/v projections hand-interleaved with the first
            # superblock's scores, following the DMA arrival order (group 1
            # feeds kqproj(0)+scores t=0,1 immediately; wv/x2e1 land a bit
            # later), so the PE never drains while chunks trickle in ----
            pt0 = []
            emit_kqproj(0, 0, 256)
            emit_scores(0, 0, pt0)
            emit_kqproj(0, 256, 512)
            emit_scores(0, 1, pt0)
            emit_kqproj(1)
            emit_vproj(0, 8)
            emit_scores(0, 2, pt0)
            emit_scores(0, 3, pt0)
            for qt in range(1, 4):
                emit_kqproj(qt * 2)
                emit_kqproj(qt * 2 + 1)
                emit_scores(0, qt * 4, pt0)
                emit_scores(0, qt * 4 + 1, pt0)
                emit_vproj(qt * 8, 8)
                emit_scores(0, qt * 4 + 2, pt0)
                emit_scores(0, qt * 4 + 3, pt0)
            emit_out(0, pt0)

            for sb in range(1, NSB):
                pt_tiles = []
                for t in range(NMM // 2):
                    emit_scores(sb, t, pt_tiles)
                emit_out(sb, pt_tiles)
    nc.compile()
    return nc


def _get_program():
    if "nc" not in _CACHE:
        _CACHE["nc"] = _build_program()
    return _CACHE["nc"]


def _pack(x, ranges):
    """[C, w] fp -> [P, KC*w] fp16 with each column-range chunk contiguous
    per partition (kc halves adjacent): one DMA descriptor per partition
    per chunk."""
    parts = []
    for a, b in ranges:
        blk = x[:, a:b].reshape(KC, P, b - a).transpose(1, 0, 2)
        parts.append(blk.reshape(P, KC * (b - a)))
    return np.ascontiguousarray(np.concatenate(parts, axis=1)
                                .astype(np.float16))


def kernel(**inputs) -> np.ndarray:
    x1 = np.asarray(inputs["x1"], np.float32).reshape(B, C, N)
    x2 = np.asarray(inputs["x2"], np.float32).reshape(B, C, N)
    x1_h = [_pack(x1[b], X1_RANGES) for b in range(B)]
    x2_h = [_pack(x2[b], X2_RANGES) for b in range(B)]
    # scores = (Wq x1)^T (Wk x2) = x1^T (Wq^T Wk) x2: fold both score
    # projections into one by shipping G = Wk^T Wq as the k-side weight;
    # x1 then feeds the score matmuls raw (saves 32 matmuls/core and one
    # rounding on the q side).
    G = (np.asarray(inputs["Wk"], np.float64).T
         @ np.asarray(inputs["Wq"], np.float64)).astype(np.float32)
    wkT = _pack(G, [(0, C)])
    wvT = _pack(np.asarray(inputs["Wv"], np.float32).T, [(0, C)])

    in_maps = [
        {"x1": x1_h[b], "x2": x2_h[b], "wkT": wkT, "wvT": wvT}
        for b in range(B)
    ]
    nc = _get_program()
    res = bass_utils.run_bass_kernel_spmd(nc, in_maps, core_ids=list(range(B)),
                                          trace=TRACE, tmpdir=TRACE_DIR)
    _CACHE["last_results"] = res
    out = np.empty((B, C, N), np.float32)
    for b in range(B):
        out[b] = res.results[b]["outT"].astype(np.float32).T
    return out.reshape(B, C, H, W)


if __name__ == "__main__":
    nc = _build_program()
    n = sum(len(b.instructions) for b in nc.m.functions[0].blocks)
    print(f"program built ok: {n} instructions")


# revision 24
# speedup vs baseline: 1.0116x; 1.0116x over previous
"""Cross-attention (1x1-conv q/k/v + softmax(Q^T K) + V@attn^T) on Trainium2.

Data-parallel over batch: 8 batches -> 8 NeuronCores, one full [N,N]
attention per core; the small CxC projection weights are replicated.

Per-core device program (all matmuls, zero transposes). The two score
projections are folded into one on the host: scores = (Wq x1)^T (Wk x2)
= x1^T G x2 with G = Wk^T Wq [CxC], so x1 feeds the score matmuls raw:
  A[c,m]   = G.T @ x2              (fp16 matmul, c on partitions)
  vT[m,c'] = x2.T @ WvT            (fp16 matmul, bf16 result; appended
                                    ones column c'=C)
  sT[m,n]  = A.T @ x1              (fp16 scores, transposed layout)
  pT[m,n]  = exp(sT - SHIFT)       (ScalarE, bf16 out; SHIFT makes per-row max
                                    subtraction unnecessary: softmax is
                                    shift-invariant and scores stay in
                                    [-150, ~110] => exp in fp32/bf16 range)
  o'[n,c'] = pT.T @ vT             (bf16; ones column accumulates row sums)
  outT[n,c] = o'[n,:C] * (1/o'[n,C])

dtype choices: the wire + score path is fp16 (e5m10). x inputs are ~N(0,1)
so fp16's range is ample, and its 10-bit mantissa matches what the PE keeps
internally for fp32r (tf32-ish) operands - measured end-to-end error is the
same as the fp32r build (~4e-3 absmax-relative) while the DMA bytes halve
and, crucially, fp16 weights get FWL + a separate LDWEIGHTS that overlaps
the previous matmul: fp32r matmuls self-load their stationary operand and
pay ~+15ns each (227-235ns vs the 216ns FD=512 roofline; fp16 reaches it).
The value path (pT, vT) is bf16 because pT = exp(s-SHIFT) spans e^-200..e^50
which overflows fp16's e^11 range. Output is written fp16 (|out| <= ~6).

The host reassembles outT -> [B, C, H, W].

Biases are not applied: the problem spec fixes bq/bk/bv to zeros.
"""

from contextlib import ExitStack

import numpy as np

import concourse.bass as bass
import concourse.mybir as mybir
import concourse.tile as tile
from concourse import bacc, bass_utils

B, C, H, W = 8, 256, 64, 64
N = H * W          # 4096 tokens per image
P = 128            # partition count
KC = C // P        # 2 contraction chunks over channels
NMM = N // P       # 32 key-side chunks
SB = 512           # query-side superblock (score matmul free dim)
NSB = N // SB      # 8
C2 = C + 1         # value width + ones column (bf16 matmuls allow odd free)
SHIFT = 60.0       # softmax exp shift (see module docstring)
NWARM = 34         # FD=128 warmup matmuls (~3.6us cold) to flip the HAM
                   # clock gate to 8/8 while the input DMA is in flight

# x chunk column ranges; host packs each chunk contiguously per partition
# (one DMA descriptor per partition per chunk instead of one per
# (partition, kc) pair - descriptor count, not bytes, limits the head).
X2_RANGES = [(0, 512), (512, 1024), (1024, 2048), (2048, 3072), (3072, 4096)]
X1_RANGES = [(0, 512), (512, 4096)]

_CACHE: dict = {}
TRACE = False       # set by test harness to capture an NTFF profile
TRACE_DIR = None    # optional fixed profile output dir


def _build_program():
    f32 = mybir.dt.float32
    f16 = mybir.dt.float16    # wire + score path
    bf16 = mybir.dt.bfloat16  # value path: range for exp(s-SHIFT)
    exp = mybir.ActivationFunctionType.Exp
    # bacc (not raw Bass): its compile() pass splits multi-semaphore waits,
    # which walrus codegen requires (one wait per TPB instruction).
    nc = bacc.Bacc("TRN2", target_bir_lowering=False, debug=False)

    # x/w arrive pre-packed: partition-major, each chunk's two kc halves
    # adjacent, so every chunk DMA is one contiguous descriptor/partition.
    x1_d = nc.dram_tensor("x1", [P, KC * N], f16, kind="ExternalInput").ap()
    x2_d = nc.dram_tensor("x2", [P, KC * N], f16, kind="ExternalInput").ap()
    wk_d = nc.dram_tensor("wkT", [P, KC * C], f16, kind="ExternalInput").ap()
    wv_d = nc.dram_tensor("wvT", [P, KC * C], f16, kind="ExternalInput").ap()
    outT_d = nc.dram_tensor("outT", [N, C], f16, kind="ExternalOutput").ap()

    with tile.TileContext(nc) as tc:
        with ExitStack() as ctx:
            consts = ctx.enter_context(tc.tile_pool(name="consts", bufs=1))
            acts = ctx.enter_context(tc.tile_pool(name="acts", bufs=1))

            # PE warmup source: memset early on the (otherwise idle) GpSimd
            # so the dummy matmuls only wait on it, not on any DMA.
            dummy = consts.tile([P, SB], f16, name="dummy")
            nc.gpsimd.memset(dummy, 0.0)

            nbias = consts.tile([P, 1], f32)
            nc.vector.memset(nbias, -SHIFT)

            # x tiles. Quarter granularity, except the first two transfers
            # are 512-col eighths so the first kq-projection + scores can
            # start as early as possible.  x1 beyond the first superblock
            # isn't needed until the steady-state loop, so it ships as one
            # big transfer at the back of the chain.
            xpool = ctx.enter_context(tc.tile_pool(name="xpool", bufs=1))
            x2_sb = [xpool.tile([P, KC, b - a], f16, name=f"x2_{a}")
                     for a, b in X2_RANGES]
            x1_sb = [xpool.tile([P, KC, b - a], f16, name=f"x1_{a}")
                     for a, b in X1_RANGES]

            def xslice(tiles, ranges, c0, c1):
                for t, (a, b) in zip(tiles, ranges):
                    if a <= c0 and c1 <= b:
                        return t[:, :, c0 - a:c1 - a]
                raise AssertionError((c0, c1))

            # DMA schedule.  The SDMA engines round-robin across all queued
            # transfers, so with no ordering everything finishes together
            # and the PE starves; but each completion->trigger handoff costs
            # ~2us of doorbell->data latency, so a fully serial chain wastes
            # ~2us per link.  Compromise: 4 priority GROUPS - members of a
            # group shard across the 16 queues concurrently, the next group
            # waits on the previous one.  Group 1 is exactly what the first
            # k-projection + first scores need.  Triggers ride on the Scalar
            # sequencer - its "main" starts ~1.2us before Sync's.
            w_sb = {nm: consts.tile([P, KC, C], f16, name=f"{nm}_sb")
                    for nm in ("wk", "wv")}

            def xsrc(src, a, b):
                return src[:, KC * a:KC * b].rearrange(
                    "p (kc w) -> p kc w", kc=KC)

            # Each group's trigger waits on an EARLY member of the previous
            # group (not full completion): the ~2.7us doorbell->data latency
            # then overlaps the previous group's in-flight tail instead of
            # stacking after it.  Triggers execute in order on the scalar
            # queue, so ordering stays monotone.
            transfers = [
                ("wk", w_sb["wk"], xsrc(wk_d, 0, C), None),
                ("x2e0", x2_sb[0], xsrc(x2_d, 0, 512), None),
                ("x1e0", x1_sb[0], xsrc(x1_d, 0, 512), "wk"),
                ("wv", w_sb["wv"], xsrc(wv_d, 0, C), "wk"),
                ("x2e1", x2_sb[1], xsrc(x2_d, 512, 1024), "x2e0"),
                ("x2q1", x2_sb[2], xsrc(x2_d, 1024, 2048), "x2e0"),
                ("x2q2", x2_sb[3], xsrc(x2_d, 2048, 3072), "x2e1"),
                ("x2q3", x2_sb[4], xsrc(x2_d, 3072, 4096), "x2e1"),
                ("x1rest", x1_sb[1], xsrc(x1_d, 512, 4096), "x2e1"),
            ]
            dmas = {}
            for nm, dst, src, dep in transfers:
                dma = nc.scalar.dma_start(out=dst, in_=src)
                if dep is not None:
                    tile.add_dep_helper(dma.ins, dmas[dep].ins,
                                        reason="dma priority group")
                dmas[nm] = dma

            # q/k as per-superblock tiles, vT per m-chunk: fine-grained deps
            # let scores/out matmuls start before all projections finish.
            k_sb = [acts.tile([P, KC, SB], f16, name=f"k_{ns}", bufs=1)
                    for ns in range(NSB)]
            vT_sb = [acts.tile([P, C2], bf16, name=f"vT_{mm}", bufs=1)
                     for mm in range(NMM)]
            for mm in range(NMM):
                nc.vector.memset(vT_sb[mm][:, C:C2], 1.0)

            # ---- pools (ps/po PSUM rotations are shared by projections
            # and the attention loop; 6 + 2 = all 8 banks) ----
            pts = ctx.enter_context(tc.tile_pool(name="pts", bufs=24))
            ps_pool = ctx.enter_context(tc.tile_pool(name="ps", bufs=3, space="PSUM"))
            po_pool = ctx.enter_context(tc.tile_pool(name="po", bufs=2, space="PSUM"))
            outp = ctx.enter_context(tc.tile_pool(name="outp", bufs=4))
            normp = ctx.enter_context(tc.tile_pool(name="normp", bufs=4))

            # ---- PE warmup: the HAM clock gate holds the PE at 1.2 GHz
            # until ~3.4us of sustained activity.  Burn that window on
            # dummy matmuls while the x DMAs are in flight so the real
            # prologue runs at 2.4 GHz.  FD=128 keeps the drain short if
            # the first data lands mid-burst.
            for wmm in range(0, NWARM, 8):
                pw = ps_pool.tile([P, 2, SB], f32, tag="ps", name=f"warm{wmm}")
                for i in range(min(8, NWARM - wmm)):
                    nc.tensor.matmul(
                        pw[:, i % 2, (i // 2) * P:(i // 2 + 1) * P],
                        lhsT=dummy[:, 0:P], rhs=dummy[:, 0:P],
                        start=True, stop=True)

            def emit_kqproj(ns, lo=0, hi=SB):
                # k_sb[ns][:, :, lo:hi] from x2 cols [ns*SB+lo, ns*SB+hi);
                # kc-outer so consecutive matmuls alternate PSUM banks
                xs = xslice(x2_sb, X2_RANGES, ns * SB + lo, ns * SB + hi)
                pq = ps_pool.tile([P, 2, hi - lo], f32, tag="ps",
                                  name=f"pq_{ns}_{lo}")
                # full-width tiles span 2 PSUM banks: kc-outer alternates
                # banks.  half-width tiles fit ONE bank: mo-outer, so the
                # first accumulation group closes before the second opens.
                loop = ([(kc, mo) for kc in range(KC) for mo in range(KC)]
                        if hi - lo == SB else
                        [(kc, mo) for mo in range(KC) for kc in range(KC)])
                for kc, mo in loop:
                    nc.tensor.matmul(
                        pq[:, mo, :],
                        lhsT=w_sb["wk"][:, kc, mo * P:(mo + 1) * P],
                        rhs=xs[:, kc, :],
                        start=(kc == 0), stop=(kc == KC - 1))
                for mo in range(KC):
                    nc.vector.tensor_copy(out=k_sb[ns][:, mo, lo:hi],
                                          in_=pq[:, mo, :])

            def emit_vproj(mm0, count):
                # m-chunks [mm0, mm0+count) of the value projection; pairs
                # of accumulators from the po rotation alternate banks
                for pr in range(count // 2):
                    pv = [po_pool.tile([P, C], f32, tag="po",
                                       name=f"pv_{mm0}_{pr}_{i}")
                          for i in range(2)]
                    for kc in range(KC):
                        for i in range(2):
                            mm = mm0 + pr * 2 + i
                            xs = xslice(x2_sb, X2_RANGES, mm * P, (mm + 1) * P)
                            nc.tensor.matmul(
                                pv[i],
                                lhsT=xs[:, kc, :],
                                rhs=w_sb["wv"][:, kc, :],
                                start=(kc == 0), stop=(kc == KC - 1))
                    for i in range(2):
                        nc.vector.tensor_copy(
                            out=vT_sb[mm0 + pr * 2 + i][:, 0:C],
                            in_=pv[i])

            def emit_scores(sb, t, pt_tiles):
                xq = xslice(x1_sb, X1_RANGES, sb * SB, (sb + 1) * SB)
                ps = ps_pool.tile([P, 2, SB], f32, tag="ps",
                                  name=f"ps_{sb}_{t}")
                for kc in range(KC):   # kc-outer: banks alternate A B A B
                    for i in range(2):
                        koff = (t * 2 + i) * P
                        kt = k_sb[koff // SB]
                        nc.tensor.matmul(
                            ps[:, i, :],
                            lhsT=kt[:, kc, koff % SB:koff % SB + P],
                            rhs=xq[:, kc, :],
                            start=(kc == 0), stop=(kc == KC - 1))
                pt = pts.tile([P, 2, SB], bf16, tag="pt")
                nc.scalar.activation(out=pt, in_=ps, func=exp,
                                     bias=nbias, scale=1.0)
                pt_tiles.append(pt)

            def emit_out(sb, pt_tiles):
                # j-outer: one live out-accumulator bank at a time.
                for j in range(SB // P):
                    po = po_pool.tile([P, C2], f32, tag="po",
                                      name=f"po_{sb}_{j}")
                    for mm in range(NMM):
                        nc.tensor.matmul(
                            po,
                            lhsT=pt_tiles[mm // 2][:, mm % 2,
                                                   j * P:(j + 1) * P],
                            rhs=vT_sb[mm],
                            start=(mm == 0), stop=(mm == NMM - 1))
                    rc = normp.tile([P, 1], f32, tag="rc")
                    nc.vector.reciprocal(rc, po[:, C:C + 1])
                    ot = outp.tile([P, C], f16, tag="ot")
                    nc.vector.tensor_scalar_mul(ot, po[:, 0:C], rc)
                    n0 = sb * SB + j * P
                    # two half-height DMAs land on two queues -> the
                    # epilogue's final transfer drains ~2x faster.
                    nc.sync.dma_start(out=outT_d[n0:n0 + P // 2, :],
                                      in_=ot[0:P // 2])
                    nc.sync.dma_start(out=outT_d[n0 + P // 2:n0 + P, :],
                                      in_=ot[P // 2:P])

            # ---- prologue: k/v projections hand-interleaved with the first
            # superblock's scores, following the DMA arrival order (group 1
            # feeds kqproj(0)+scores t=0,1 immediately; wv/x2e1 land a bit
            # later), so the PE never drains while chunks trickle in ----
            pt0 = []
            emit_kqproj(0)
            emit_scores(0, 0, pt0)
            emit_scores(0, 1, pt0)
            emit_kqproj(1)
            emit_vproj(0, 8)
            emit_scores(0, 2, pt0)
            emit_scores(0, 3, pt0)
            for qt in range(1, 4):
                emit_kqproj(qt * 2)
                emit_kqproj(qt * 2 + 1)
                emit_scores(0, qt * 4, pt0)
                emit_scores(0, qt * 4 + 1, pt0)
                emit_vproj(qt * 8, 8)
                emit_scores(0, qt * 4 + 2, pt0)
                emit_scores(0, qt * 4 + 3, pt0)
            emit_out(0, pt0)

            for sb in range(1, NSB):
                pt_tiles = []
                for t in range(NMM // 2):
                    emit_scores(sb, t, pt_tiles)
                emit_out(sb, pt_tiles)
    nc.compile()
    return nc


def _get_program():
    if "nc" not in _CACHE:
        _CACHE["nc"] = _build_program()
    return _CACHE["nc"]


def _pack(x, ranges):
    """[C, w] fp -> [P, KC*w] fp16 with each column-range chunk contiguous
    per partition (kc halves adjacent): one DMA descriptor per partition
    per chunk."""
    parts = []
    for a, b in ranges:
        blk = x[:, a:b].reshape(KC, P, b - a).transpose(1, 0, 2)
        parts.append(blk.reshape(P, KC * (b - a)))
    return np.ascontiguousarray(np.concatenate(parts, axis=1)
                                .astype(np.float16))


def kernel(**inputs) -> np.ndarray:
    x1 = np.asarray(inputs["x1"], np.float32).reshape(B, C, N)
    x2 = np.asarray(inputs["x2"], np.float32).reshape(B, C, N)
    x1_h = [_pack(x1[b], X1_RANGES) for b in range(B)]
    x2_h = [_pack(x2[b], X2_RANGES) for b in range(B)]
    # scores = (Wq x1)^T (Wk x2) = x1^T (Wq^T Wk) x2: fold both score
    # projections into one by shipping G = Wk^T Wq as the k-side weight;
    # x1 then feeds the score matmuls raw (saves 32 matmuls/core and one
    # rounding on the q side).
    G = (np.asarray(inputs["Wk"], np.float64).T
         @ np.asarray(inputs["Wq"], np.float64)).astype(np.float32)
    wkT = _pack(G, [(0, C)])
    wvT = _pack(np.asarray(inputs["Wv"], np.float32).T, [(0, C)])

    in_maps = [
        {"x1": x1_h[b], "x2": x2_h[b], "wkT": wkT, "wvT": wvT}
        for b in range(B)
    ]
    nc = _get_program()
    res = bass_utils.run_bass_kernel_spmd(nc, in_maps, core_ids=list(range(B)),
                                          trace=TRACE, tmpdir=TRACE_DIR)
    _CACHE["last_results"] = res
    out = np.empty((B, C, N), np.float32)
    for b in range(B):
        out[b] = res.results[b]["outT"].astype(np.float32).T
    return out.reshape(B, C, H, W)


if __name__ == "__main__":
    nc = _build_program()
    n = sum(len(b.instructions) for b in nc.m.functions[0].blocks)
    print(f"program built ok: {n} instructions")


# revision 26
# speedup vs baseline: 1.0123x; 1.0007x over previous
"""Cross-attention (1x1-conv q/k/v + softmax(Q^T K) + V@attn^T) on Trainium2.

Data-parallel over batch: 8 batches -> 8 NeuronCores, one full [N,N]
attention per core; the small CxC projection weights are replicated.

Per-core device program (all matmuls, zero transposes). The two score
projections are folded into one on the host: scores = (Wq x1)^T (Wk x2)
= x1^T G x2 with G = Wk^T Wq [CxC], so x1 feeds the score matmuls raw:
  A[c,m]   = G.T @ x2              (fp16 matmul, c on partitions)
  vT[m,c'] = x2.T @ WvT            (fp16 matmul, bf16 result; appended
                                    ones column c'=C)
  sT[m,n]  = A.T @ x1              (fp16 scores, transposed layout)
  pT[m,n]  = exp(sT - SHIFT)       (ScalarE, bf16 out; SHIFT makes per-row max
                                    subtraction unnecessary: softmax is
                                    shift-invariant and scores stay in
                                    [-150, ~110] => exp in fp32/bf16 range)
  o'[n,c'] = pT.T @ vT             (bf16; ones column accumulates row sums)
  outT[n,c] = o'[n,:C] * (1/o'[n,C])

dtype choices: the wire + score path is fp16 (e5m10). x inputs are ~N(0,1)
so fp16's range is ample, and its 10-bit mantissa matches what the PE keeps
internally for fp32r (tf32-ish) operands - measured end-to-end error is the
same as the fp32r build (~4e-3 absmax-relative) while the DMA bytes halve
and, crucially, fp16 weights get FWL + a separate LDWEIGHTS that overlaps
the previous matmul: fp32r matmuls self-load their stationary operand and
pay ~+15ns each (227-235ns vs the 216ns FD=512 roofline; fp16 reaches it).
The value path (pT, vT) is bf16 because pT = exp(s-SHIFT) spans e^-200..e^50
which overflows fp16's e^11 range. Output is written fp16 (|out| <= ~6).

The host reassembles outT -> [B, C, H, W].

Biases are not applied: the problem spec fixes bq/bk/bv to zeros.
"""

from contextlib import ExitStack

import numpy as np

import concourse.bass as bass
import concourse.mybir as mybir
import concourse.tile as tile
from concourse import bacc, bass_utils

B, C, H, W = 8, 256, 64, 64
N = H * W          # 4096 tokens per image
P = 128            # partition count
KC = C // P        # 2 contraction chunks over channels
NMM = N // P       # 32 key-side chunks
SB = 512           # query-side superblock (score matmul free dim)
NSB = N // SB      # 8
C2 = C + 1         # value width + ones column (bf16 matmuls allow odd free)
SHIFT = 60.0       # softmax exp shift (see module docstring)
NWARM = 44         # FD=128 warmup matmuls (~4.7us cold) to flip the HAM
                   # clock gate to 8/8 while the input DMA is in flight; sized
                   # to end right when group-1 data becomes consumable

# x chunk column ranges; host packs each chunk contiguously per partition
# (one DMA descriptor per partition per chunk instead of one per
# (partition, kc) pair - descriptor count, not bytes, limits the head).
X2_RANGES = [(0, 512), (512, 1024), (1024, 2048), (2048, 3072), (3072, 4096)]
X1_RANGES = [(0, 512), (512, 4096)]

_CACHE: dict = {}
TRACE = False       # set by test harness to capture an NTFF profile
TRACE_DIR = None    # optional fixed profile output dir


def _build_program():
    f32 = mybir.dt.float32
    f16 = mybir.dt.float16    # wire + score path
    bf16 = mybir.dt.bfloat16  # value path: range for exp(s-SHIFT)
    exp = mybir.ActivationFunctionType.Exp
    # bacc (not raw Bass): its compile() pass splits multi-semaphore waits,
    # which walrus codegen requires (one wait per TPB instruction).
    nc = bacc.Bacc("TRN2", target_bir_lowering=False, debug=False)

    # x/w arrive pre-packed: partition-major, each chunk's two kc halves
    # adjacent, so every chunk DMA is one contiguous descriptor/partition.
    x1_d = nc.dram_tensor("x1", [P, KC * N], f16, kind="ExternalInput").ap()
    x2_d = nc.dram_tensor("x2", [P, KC * N], f16, kind="ExternalInput").ap()
    wk_d = nc.dram_tensor("wkT", [P, KC * C], f16, kind="ExternalInput").ap()
    wv_d = nc.dram_tensor("wvT", [P, KC * C], f16, kind="ExternalInput").ap()
    outT_d = nc.dram_tensor("outT", [N, C], f16, kind="ExternalOutput").ap()

    with tile.TileContext(nc) as tc:
        with ExitStack() as ctx:
            consts = ctx.enter_context(tc.tile_pool(name="consts", bufs=1))
            acts = ctx.enter_context(tc.tile_pool(name="acts", bufs=1))

            # PE warmup source: memset early on the (otherwise idle) GpSimd
            # so the dummy matmuls only wait on it, not on any DMA.
            dummy = consts.tile([P, SB], f16, name="dummy")
            nc.gpsimd.memset(dummy, 0.0)

            nbias = consts.tile([P, 1], f32)
            nc.vector.memset(nbias, -SHIFT)

            # x tiles. Quarter granularity, except the first two transfers
            # are 512-col eighths so the first kq-projection + scores can
            # start as early as possible.  x1 beyond the first superblock
            # isn't needed until the steady-state loop, so it ships as one
            # big transfer at the back of the chain.
            xpool = ctx.enter_context(tc.tile_pool(name="xpool", bufs=1))
            x2_sb = [xpool.tile([P, KC, b - a], f16, name=f"x2_{a}")
                     for a, b in X2_RANGES]
            x1_sb = [xpool.tile([P, KC, b - a], f16, name=f"x1_{a}")
                     for a, b in X1_RANGES]

            def xslice(tiles, ranges, c0, c1):
                for t, (a, b) in zip(tiles, ranges):
                    if a <= c0 and c1 <= b:
                        return t[:, :, c0 - a:c1 - a]
                raise AssertionError((c0, c1))

            # DMA schedule.  The SDMA engines round-robin across all queued
            # transfers, so with no ordering everything finishes together
            # and the PE starves; but each completion->trigger handoff costs
            # ~2us of doorbell->data latency, so a fully serial chain wastes
            # ~2us per link.  Compromise: 4 priority GROUPS - members of a
            # group shard across the 16 queues concurrently, the next group
            # waits on the previous one.  Group 1 is exactly what the first
            # k-projection + first scores need.  Triggers ride on the Scalar
            # sequencer - its "main" starts ~1.2us before Sync's.
            w_sb = {nm: consts.tile([P, KC, C], f16, name=f"{nm}_sb")
                    for nm in ("wk", "wv")}

            def xsrc(src, a, b):
                return src[:, KC * a:KC * b].rearrange(
                    "p (kc w) -> p kc w", kc=KC)

            # Each group's trigger waits on an EARLY member of the previous
            # group (not full completion): the ~2.7us doorbell->data latency
            # then overlaps the previous group's in-flight tail instead of
            # stacking after it.  Triggers execute in order on the scalar
            # queue, so ordering stays monotone.
            transfers = [
                ("wk", w_sb["wk"], xsrc(wk_d, 0, C), None),
                ("x2e0", x2_sb[0], xsrc(x2_d, 0, 512), None),
                ("x1e0", x1_sb[0], xsrc(x1_d, 0, 512), None),
                ("wv", w_sb["wv"], xsrc(wv_d, 0, C), "wk"),
                ("x2e1", x2_sb[1], xsrc(x2_d, 512, 1024), "wk"),
                ("x2q1", x2_sb[2], xsrc(x2_d, 1024, 2048), "x2e0"),
                ("x2q2", x2_sb[3], xsrc(x2_d, 2048, 3072), "x2e1"),
                ("x2q3", x2_sb[4], xsrc(x2_d, 3072, 4096), "x2e1"),
                ("x1rest", x1_sb[1], xsrc(x1_d, 512, 4096), "x2e1"),
            ]
            dmas = {}
            for nm, dst, src, dep in transfers:
                dma = nc.scalar.dma_start(out=dst, in_=src)
                if dep is not None:
                    tile.add_dep_helper(dma.ins, dmas[dep].ins,
                                        reason="dma priority group")
                dmas[nm] = dma

            # q/k as per-superblock tiles, vT per m-chunk: fine-grained deps
            # let scores/out matmuls start before all projections finish.
            k_sb = [acts.tile([P, KC, SB], f16, name=f"k_{ns}", bufs=1)
                    for ns in range(NSB)]
            vT_sb = [acts.tile([P, C2], bf16, name=f"vT_{mm}", bufs=1)
                     for mm in range(NMM)]
            for mm in range(NMM):
                nc.vector.memset(vT_sb[mm][:, C:C2], 1.0)

            # ---- pools (ps/po PSUM rotations are shared by projections
            # and the attention loop; 6 + 2 = all 8 banks) ----
            pts = ctx.enter_context(tc.tile_pool(name="pts", bufs=24))
            ps_pool = ctx.enter_context(tc.tile_pool(name="ps", bufs=3, space="PSUM"))
            po_pool = ctx.enter_context(tc.tile_pool(name="po", bufs=2, space="PSUM"))
            outp = ctx.enter_context(tc.tile_pool(name="outp", bufs=4))
            normp = ctx.enter_context(tc.tile_pool(name="normp", bufs=4))

            # ---- PE warmup: the HAM clock gate holds the PE at 1.2 GHz
            # until ~3.4us of sustained activity.  Burn that window on
            # dummy matmuls while the x DMAs are in flight so the real
            # prologue runs at 2.4 GHz.  FD=128 keeps the drain short if
            # the first data lands mid-burst.
            for wmm in range(0, NWARM, 8):
                pw = ps_pool.tile([P, 2, SB], f32, tag="ps", name=f"warm{wmm}")
                for i in range(min(8, NWARM - wmm)):
                    nc.tensor.matmul(
                        pw[:, i % 2, (i // 2) * P:(i // 2 + 1) * P],
                        lhsT=dummy[:, 0:P], rhs=dummy[:, 0:P],
                        start=True, stop=True)

            def emit_kqproj(ns, lo=0, hi=SB):
                # k_sb[ns][:, :, lo:hi] from x2 cols [ns*SB+lo, ns*SB+hi);
                # kc-outer so consecutive matmuls alternate PSUM banks
                xs = xslice(x2_sb, X2_RANGES, ns * SB + lo, ns * SB + hi)
                pq = ps_pool.tile([P, 2, hi - lo], f32, tag="ps",
                                  name=f"pq_{ns}_{lo}")
                # full-width tiles span 2 PSUM banks: kc-outer alternates
                # banks.  half-width tiles fit ONE bank: mo-outer, so the
                # first accumulation group closes before the second opens.
                loop = ([(kc, mo) for kc in range(KC) for mo in range(KC)]
                        if hi - lo == SB else
                        [(kc, mo) for mo in range(KC) for kc in range(KC)])
                for kc, mo in loop:
                    nc.tensor.matmul(
                        pq[:, mo, :],
                        lhsT=w_sb["wk"][:, kc, mo * P:(mo + 1) * P],
                        rhs=xs[:, kc, :],
                        start=(kc == 0), stop=(kc == KC - 1))
                for mo in range(KC):
                    nc.vector.tensor_copy(out=k_sb[ns][:, mo, lo:hi],
                                          in_=pq[:, mo, :])

            def emit_vproj(mm0, count):
                # m-chunks [mm0, mm0+count) of the value projection; pairs
                # of accumulators from the po rotation alternate banks
                for pr in range(count // 2):
                    pv = [po_pool.tile([P, C], f32, tag="po",
                                       name=f"pv_{mm0}_{pr}_{i}")
                          for i in range(2)]
                    for kc in range(KC):
                        for i in range(2):
                            mm = mm0 + pr * 2 + i
                            xs = xslice(x2_sb, X2_RANGES, mm * P, (mm + 1) * P)
                            nc.tensor.matmul(
                                pv[i],
                                lhsT=xs[:, kc, :],
                                rhs=w_sb["wv"][:, kc, :],
                                start=(kc == 0), stop=(kc == KC - 1))
                    for i in range(2):
                        nc.vector.tensor_copy(
                            out=vT_sb[mm0 + pr * 2 + i][:, 0:C],
                            in_=pv[i])

            def emit_scores(sb, t, pt_tiles):
                xq = xslice(x1_sb, X1_RANGES, sb * SB, (sb + 1) * SB)
                ps = ps_pool.tile([P, 2, SB], f32, tag="ps",
                                  name=f"ps_{sb}_{t}")
                for kc in range(KC):   # kc-outer: banks alternate A B A B
                    for i in range(2):
                        koff = (t * 2 + i) * P
                        kt = k_sb[koff // SB]
                        nc.tensor.matmul(
                            ps[:, i, :],
                            lhsT=kt[:, kc, koff % SB:koff % SB + P],
                            rhs=xq[:, kc, :],
                            start=(kc == 0), stop=(kc == KC - 1))
                pt = pts.tile([P, 2, SB], bf16, tag="pt")
                nc.scalar.activation(out=pt, in_=ps, func=exp,
                                     bias=nbias, scale=1.0)
                pt_tiles.append(pt)

            def emit_out(sb, pt_tiles):
                # j-outer: one live out-accumulator bank at a time.
                for j in range(SB // P):
                    po = po_pool.tile([P, C2], f32, tag="po",
                                      name=f"po_{sb}_{j}")
                    for mm in range(NMM):
                        nc.tensor.matmul(
                            po,
                            lhsT=pt_tiles[mm // 2][:, mm % 2,
                                                   j * P:(j + 1) * P],
                            rhs=vT_sb[mm],
                            start=(mm == 0), stop=(mm == NMM - 1))
                    rc = normp.tile([P, 1], f32, tag="rc")
                    nc.vector.reciprocal(rc, po[:, C:C + 1])
                    ot = outp.tile([P, C], f16, tag="ot")
                    nc.vector.tensor_scalar_mul(ot, po[:, 0:C], rc)
                    n0 = sb * SB + j * P
                    # two half-height DMAs land on two queues -> the
                    # epilogue's final transfer drains ~2x faster.
                    nc.sync.dma_start(out=outT_d[n0:n0 + P // 2, :],
                                      in_=ot[0:P // 2])
                    nc.sync.dma_start(out=outT_d[n0 + P // 2:n0 + P, :],
                                      in_=ot[P // 2:P])

            # ---- prologue: k/v projections hand-interleaved with the first
            # superblock's scores, following the DMA arrival order (group 1
            # feeds kqproj(0)+scores t=0,1 immediately; wv/x2e1 land a bit
            # later), so the PE never drains while chunks trickle in ----
            pt0 = []
            emit_kqproj(0)
            emit_scores(0, 0, pt0)
            emit_scores(0, 1, pt0)
            emit_kqproj(1)
            emit_vproj(0, 8)
            emit_scores(0, 2, pt0)
            emit_scores(0, 3, pt0)
            for qt in range(1, 4):
                emit_kqproj(qt * 2)
                emit_kqproj(qt * 2 + 1)
                emit_scores(0, qt * 4, pt0)
                emit_scores(0, qt * 4 + 1, pt0)
                emit_vproj(qt * 8, 8)
                emit_scores(0, qt * 4 + 2, pt0)
                emit_scores(0, qt * 4 + 3, pt0)
            emit_out(0, pt0)

            for sb in range(1, NSB):
                pt_tiles = []
                for t in range(NMM // 2):
                    emit_scores(sb, t, pt_tiles)
                emit_out(sb, pt_tiles)
    nc.compile()
    return nc


def _get_program():
    if "nc" not in _CACHE:
        _CACHE["nc"] = _build_program()
    return _CACHE["nc"]


def _pack(x, ranges):
    """[C, w] fp -> [P, KC*w] fp16 with each column-range chunk contiguous
    per partition (kc halves adjacent): one DMA descriptor per partition
    per chunk."""
    parts = []
    for a, b in ranges:
        blk = x[:, a:b].reshape(KC, P, b - a).transpose(1, 0, 2)
        parts.append(blk.reshape(P, KC * (b - a)))
    return np.ascontiguousarray(np.concatenate(parts, axis=1)
                                .astype(np.float16))


def kernel(**inputs) -> np.ndarray:
    x1 = np.asarray(inputs["x1"], np.float32).reshape(B, C, N)
    x2 = np.asarray(inputs["x2"], np.float32).reshape(B, C, N)
    x1_h = [_pack(x1[b], X1_RANGES) for b in range(B)]
    x2_h = [_pack(x2[b], X2_RANGES) for b in range(B)]
    # scores = (Wq x1)^T (Wk x2) = x1^T (Wq^T Wk) x2: fold both score
    # projections into one by shipping G = Wk^T Wq as the k-side weight;
    # x1 then feeds the score matmuls raw (saves 32 matmuls/core and one
    # rounding on the q side).
    G = (np.asarray(inputs["Wk"], np.float64).T
         @ np.asarray(inputs["Wq"], np.float64)).astype(np.float32)
    wkT = _pack(G, [(0, C)])
    wvT = _pack(np.asarray(inputs["Wv"], np.float32).T, [(0, C)])

    in_maps = [
        {"x1": x1_h[b], "x2": x2_h[b], "wkT": wkT, "wvT": wvT}
        for b in range(B)
    ]
    nc = _get_program()
    res = bass_utils.run_bass_kernel_spmd(nc, in_maps, core_ids=list(range(B)),
                                          trace=TRACE, tmpdir=TRACE_DIR)
    _CACHE["last_results"] = res
    out = np.empty((B, C, N), np.float32)
    for b in range(B):
        out[b] = res.results[b]["outT"].astype(np.float32).T
    return out.reshape(B, C, H, W)


if __name__ == "__main__":
    nc = _build_program()
    n = sum(len(b.instructions) for b in nc.m.functions[0].blocks)
    print(f"program built ok: {n} instructions")


# revision 27
# speedup vs baseline: 1.0200x; 1.0076x over previous
"""Cross-attention (1x1-conv q/k/v + softmax(Q^T K) + V@attn^T) on Trainium2.

Data-parallel over batch: 8 batches -> 8 NeuronCores, one full [N,N]
attention per core; the small CxC projection weights are replicated.

Per-core device program (all matmuls, zero transposes). The two score
projections are folded into one on the host: scores = (Wq x1)^T (Wk x2)
= x1^T G x2 with G = Wk^T Wq [CxC], so x1 feeds the score matmuls raw:
  A[c,m]   = G.T @ x2              (fp16 matmul, c on partitions)
  vT[m,c'] = x2.T @ WvT            (fp16 matmul, bf16 result; appended
                                    ones column c'=C)
  sT[m,n]  = A.T @ x1              (fp16 scores, transposed layout)
  pT[m,n]  = exp(sT - SHIFT)       (ScalarE, bf16 out; SHIFT makes per-row max
                                    subtraction unnecessary: softmax is
                                    shift-invariant and scores stay in
                                    [-150, ~110] => exp in fp32/bf16 range)
  o'[n,c'] = pT.T @ vT             (bf16; ones column accumulates row sums)
  outT[n,c] = o'[n,:C] * (1/o'[n,C])

dtype choices: the wire + score path is fp16 (e5m10). x inputs are ~N(0,1)
so fp16's range is ample, and its 10-bit mantissa matches what the PE keeps
internally for fp32r (tf32-ish) operands - measured end-to-end error is the
same as the fp32r build (~4e-3 absmax-relative) while the DMA bytes halve
and, crucially, fp16 weights get FWL + a separate LDWEIGHTS that overlaps
the previous matmul: fp32r matmuls self-load their stationary operand and
pay ~+15ns each (227-235ns vs the 216ns FD=512 roofline; fp16 reaches it).
The value path (pT, vT) is bf16 because pT = exp(s-SHIFT) spans e^-200..e^50
which overflows fp16's e^11 range. Output is written fp16 (|out| <= ~6).

The host reassembles outT -> [B, C, H, W].

Biases are not applied: the problem spec fixes bq/bk/bv to zeros.
"""

from contextlib import ExitStack

import numpy as np

import concourse.bass as bass
import concourse.mybir as mybir
import concourse.tile as tile
from concourse import bacc, bass_utils

B, C, H, W = 8, 256, 64, 64
N = H * W          # 4096 tokens per image
P = 128            # partition count
KC = C // P        # 2 contraction chunks over channels
NMM = N // P       # 32 key-side chunks
SB = 512           # query-side superblock (score matmul free dim)
NSB = N // SB      # 8
C2 = C + 1         # value width + ones column (bf16 matmuls allow odd free)
SHIFT = 60.0       # softmax exp shift (see module docstring)
NWARM = 44         # FD=128 warmup matmuls (~4.7us cold) to flip the HAM
                   # clock gate to 8/8 while the input DMA is in flight; sized
                   # to end right when group-1 data becomes consumable

# x chunk column ranges; host packs each chunk contiguously per partition
# (one DMA descriptor per partition per chunk instead of one per
# (partition, kc) pair - descriptor count, not bytes, limits the head).
X2_RANGES = [(0, 512), (512, 1024), (1024, 2048), (2048, 3072), (3072, 4096)]
X1_RANGES = [(0, 512), (512, 4096)]

_CACHE: dict = {}
TRACE = False       # set by test harness to capture an NTFF profile
TRACE_DIR = None    # optional fixed profile output dir


def _build_program():
    f32 = mybir.dt.float32
    f16 = mybir.dt.float16    # wire + score path
    bf16 = mybir.dt.bfloat16  # value path: range for exp(s-SHIFT)
    exp = mybir.ActivationFunctionType.Exp
    # bacc (not raw Bass): its compile() pass splits multi-semaphore waits,
    # which walrus codegen requires (one wait per TPB instruction).
    nc = bacc.Bacc("TRN2", target_bir_lowering=False, debug=False)

    # x/w arrive pre-packed: partition-major, each chunk's two kc halves
    # adjacent, so every chunk DMA is one contiguous descriptor/partition.
    x1_d = nc.dram_tensor("x1", [P, KC * N], f16, kind="ExternalInput").ap()
    x2_d = nc.dram_tensor("x2", [P, KC * N], f16, kind="ExternalInput").ap()
    wk_d = nc.dram_tensor("wkT", [P, KC * C], f16, kind="ExternalInput").ap()
    wv_d = nc.dram_tensor("wvT", [P, KC * C], f16, kind="ExternalInput").ap()
    outT_d = nc.dram_tensor("outT", [N, C], f16, kind="ExternalOutput").ap()

    with tile.TileContext(nc) as tc:
        with ExitStack() as ctx:
            consts = ctx.enter_context(tc.tile_pool(name="consts", bufs=1))
            acts = ctx.enter_context(tc.tile_pool(name="acts", bufs=1))

            # PE warmup source: memset early on the (otherwise idle) GpSimd
            # so the dummy matmuls only wait on it, not on any DMA.
            dummy = consts.tile([P, SB], f16, name="dummy")
            nc.gpsimd.memset(dummy, 0.0)

            nbias = consts.tile([P, 1], f32)
            nc.vector.memset(nbias, -SHIFT)

            # x tiles. Quarter granularity, except the first two transfers
            # are 512-col eighths so the first kq-projection + scores can
            # start as early as possible.  x1 beyond the first superblock
            # isn't needed until the steady-state loop, so it ships as one
            # big transfer at the back of the chain.
            xpool = ctx.enter_context(tc.tile_pool(name="xpool", bufs=1))
            x2_sb = [xpool.tile([P, KC, b - a], f16, name=f"x2_{a}")
                     for a, b in X2_RANGES]
            x1_sb = [xpool.tile([P, KC, b - a], f16, name=f"x1_{a}")
                     for a, b in X1_RANGES]

            def xslice(tiles, ranges, c0, c1):
                for t, (a, b) in zip(tiles, ranges):
                    if a <= c0 and c1 <= b:
                        return t[:, :, c0 - a:c1 - a]
                raise AssertionError((c0, c1))

            # DMA schedule.  The SDMA engines round-robin across all queued
            # transfers, so with no ordering everything finishes together
            # and the PE starves; but each completion->trigger handoff costs
            # ~2us of doorbell->data latency, so a fully serial chain wastes
            # ~2us per link.  Compromise: 4 priority GROUPS - members of a
            # group shard across the 16 queues concurrently, the next group
            # waits on the previous one.  Group 1 is exactly what the first
            # k-projection + first scores need.  Triggers ride on the Scalar
            # sequencer - its "main" starts ~1.2us before Sync's.
            w_sb = {nm: consts.tile([P, KC, C], f16, name=f"{nm}_sb")
                    for nm in ("wk", "wv")}

            def xsrc(src, a, b):
                return src[:, KC * a:KC * b].rearrange(
                    "p (kc w) -> p kc w", kc=KC)

            # Each group's trigger waits on an EARLY member of the previous
            # group (not full completion): the ~2.7us doorbell->data latency
            # then overlaps the previous group's in-flight tail instead of
            # stacking after it.  Triggers execute in order on the scalar
            # queue, so ordering stays monotone.
            transfers = [
                ("wk", w_sb["wk"], xsrc(wk_d, 0, C), None),
                ("x2e0", x2_sb[0], xsrc(x2_d, 0, 512), None),
                ("x1e0", x1_sb[0], xsrc(x1_d, 0, 512), None),
                ("wv", w_sb["wv"], xsrc(wv_d, 0, C), None),
                ("x2e1", x2_sb[1], xsrc(x2_d, 512, 1024), None),
                ("x2q1", x2_sb[2], xsrc(x2_d, 1024, 2048), None),
                ("x2q2", x2_sb[3], xsrc(x2_d, 2048, 3072), None),
                ("x2q3", x2_sb[4], xsrc(x2_d, 3072, 4096), None),
                ("x1rest", x1_sb[1], xsrc(x1_d, 512, 4096), None),
            ]
            dmas = {}
            for nm, dst, src, dep in transfers:
                dma = nc.scalar.dma_start(out=dst, in_=src)
                if dep is not None:
                    tile.add_dep_helper(dma.ins, dmas[dep].ins,
                                        reason="dma priority group")
                dmas[nm] = dma

            # q/k as per-superblock tiles, vT per m-chunk: fine-grained deps
            # let scores/out matmuls start before all projections finish.
            k_sb = [acts.tile([P, KC, SB], f16, name=f"k_{ns}", bufs=1)
                    for ns in range(NSB)]
            vT_sb = [acts.tile([P, C2], bf16, name=f"vT_{mm}", bufs=1)
                     for mm in range(NMM)]
            for mm in range(NMM):
                nc.vector.memset(vT_sb[mm][:, C:C2], 1.0)

            # ---- pools (ps/po PSUM rotations are shared by projections
            # and the attention loop; 6 + 2 = all 8 banks) ----
            pts = ctx.enter_context(tc.tile_pool(name="pts", bufs=24))
            ps_pool = ctx.enter_context(tc.tile_pool(name="ps", bufs=3, space="PSUM"))
            po_pool = ctx.enter_context(tc.tile_pool(name="po", bufs=2, space="PSUM"))
            outp = ctx.enter_context(tc.tile_pool(name="outp", bufs=4))
            normp = ctx.enter_context(tc.tile_pool(name="normp", bufs=4))

            # ---- PE warmup: the HAM clock gate holds the PE at 1.2 GHz
            # until ~3.4us of sustained activity.  Burn that window on
            # dummy matmuls while the x DMAs are in flight so the real
            # prologue runs at 2.4 GHz.  FD=128 keeps the drain short if
            # the first data lands mid-burst.
            for wmm in range(0, NWARM, 8):
                pw = ps_pool.tile([P, 2, SB], f32, tag="ps", name=f"warm{wmm}")
                for i in range(min(8, NWARM - wmm)):
                    nc.tensor.matmul(
                        pw[:, i % 2, (i // 2) * P:(i // 2 + 1) * P],
                        lhsT=dummy[:, 0:P], rhs=dummy[:, 0:P],
                        start=True, stop=True)

            def emit_kqproj(ns, lo=0, hi=SB):
                # k_sb[ns][:, :, lo:hi] from x2 cols [ns*SB+lo, ns*SB+hi);
                # kc-outer so consecutive matmuls alternate PSUM banks
                xs = xslice(x2_sb, X2_RANGES, ns * SB + lo, ns * SB + hi)
                pq = ps_pool.tile([P, 2, hi - lo], f32, tag="ps",
                                  name=f"pq_{ns}_{lo}")
                # full-width tiles span 2 PSUM banks: kc-outer alternates
                # banks.  half-width tiles fit ONE bank: mo-outer, so the
                # first accumulation group closes before the second opens.
                loop = ([(kc, mo) for kc in range(KC) for mo in range(KC)]
                        if hi - lo == SB else
                        [(kc, mo) for mo in range(KC) for kc in range(KC)])
                for kc, mo in loop:
                    nc.tensor.matmul(
                        pq[:, mo, :],
                        lhsT=w_sb["wk"][:, kc, mo * P:(mo + 1) * P],
                        rhs=xs[:, kc, :],
                        start=(kc == 0), stop=(kc == KC - 1))
                for mo in range(KC):
                    nc.vector.tensor_copy(out=k_sb[ns][:, mo, lo:hi],
                                          in_=pq[:, mo, :])

            def emit_vproj(mm0, count):
                # m-chunks [mm0, mm0+count) of the value projection; pairs
                # of accumulators from the po rotation alternate banks
                for pr in range(count // 2):
                    pv = [po_pool.tile([P, C], f32, tag="po",
                                       name=f"pv_{mm0}_{pr}_{i}")
                          for i in range(2)]
                    for kc in range(KC):
                        for i in range(2):
                            mm = mm0 + pr * 2 + i
                            xs = xslice(x2_sb, X2_RANGES, mm * P, (mm + 1) * P)
                            nc.tensor.matmul(
                                pv[i],
                                lhsT=xs[:, kc, :],
                                rhs=w_sb["wv"][:, kc, :],
                                start=(kc == 0), stop=(kc == KC - 1))
                    for i in range(2):
                        nc.vector.tensor_copy(
                            out=vT_sb[mm0 + pr * 2 + i][:, 0:C],
                            in_=pv[i])

            def emit_scores(sb, t, pt_tiles):
                xq = xslice(x1_sb, X1_RANGES, sb * SB, (sb + 1) * SB)
                ps = ps_pool.tile([P, 2, SB], f32, tag="ps",
                                  name=f"ps_{sb}_{t}")
                for kc in range(KC):   # kc-outer: banks alternate A B A B
                    for i in range(2):
                        koff = (t * 2 + i) * P
                        kt = k_sb[koff // SB]
                        nc.tensor.matmul(
                            ps[:, i, :],
                            lhsT=kt[:, kc, koff % SB:koff % SB + P],
                            rhs=xq[:, kc, :],
                            start=(kc == 0), stop=(kc == KC - 1))
                pt = pts.tile([P, 2, SB], bf16, tag="pt")
                nc.scalar.activation(out=pt, in_=ps, func=exp,
                                     bias=nbias, scale=1.0)
                pt_tiles.append(pt)

            def emit_out(sb, pt_tiles):
                # j-outer: one live out-accumulator bank at a time.
                for j in range(SB // P):
                    po = po_pool.tile([P, C2], f32, tag="po",
                                      name=f"po_{sb}_{j}")
                    for mm in range(NMM):
                        nc.tensor.matmul(
                            po,
                            lhsT=pt_tiles[mm // 2][:, mm % 2,
                                                   j * P:(j + 1) * P],
                            rhs=vT_sb[mm],
                            start=(mm == 0), stop=(mm == NMM - 1))
                    rc = normp.tile([P, 1], f32, tag="rc")
                    nc.vector.reciprocal(rc, po[:, C:C + 1])
                    ot = outp.tile([P, C], f16, tag="ot")
                    nc.vector.tensor_scalar_mul(ot, po[:, 0:C], rc)
                    n0 = sb * SB + j * P
                    # two half-height DMAs land on two queues -> the
                    # epilogue's final transfer drains ~2x faster.
                    nc.sync.dma_start(out=outT_d[n0:n0 + P // 2, :],
                                      in_=ot[0:P // 2])
                    nc.sync.dma_start(out=outT_d[n0 + P // 2:n0 + P, :],
                                      in_=ot[P // 2:P])

            # ---- prologue: k/v projections hand-interleaved with the first
            # superblock's scores, following the DMA arrival order (group 1
            # feeds kqproj(0)+scores t=0,1 immediately; wv/x2e1 land a bit
            # later), so the PE never drains while chunks trickle in ----
            pt0 = []
            emit_kqproj(0)
            emit_scores(0, 0, pt0)
            emit_scores(0, 1, pt0)
            emit_kqproj(1)
            emit_vproj(0, 8)
            emit_scores(0, 2, pt0)
            emit_scores(0, 3, pt0)
            for qt in range(1, 4):
                emit_kqproj(qt * 2)
                emit_kqproj(qt * 2 + 1)
                emit_scores(0, qt * 4, pt0)
                emit_scores(0, qt * 4 + 1, pt0)
                emit_vproj(qt * 8, 8)
                emit_scores(0, qt * 4 + 2, pt0)
                emit_scores(0, qt * 4 + 3, pt0)
            emit_out(0, pt0)

            for sb in range(1, NSB):
                pt_tiles = []
                for t in range(NMM // 2):
                    emit_scores(sb, t, pt_tiles)
                emit_out(sb, pt_tiles)
    nc.compile()
    return nc


def _get_program():
    if "nc" not in _CACHE:
        _CACHE["nc"] = _build_program()
    return _CACHE["nc"]


def _pack(x, ranges):
    """[C, w] fp -> [P, KC*w] fp16 with each column-range chunk contiguous
    per partition (kc halves adjacent): one DMA descriptor per partition
    per chunk."""
    parts = []
    for a, b in ranges:
        blk = x[:, a:b].reshape(KC, P, b - a).transpose(1, 0, 2)
        parts.append(blk.reshape(P, KC * (b - a)))
    return np.ascontiguousarray(np.concatenate(parts, axis=1)
                                .astype(np.float16))


def kernel(**inputs) -> np.ndarray:
    x1 = np.asarray(inputs["x1"], np.float32).reshape(B, C, N)
    x2 = np.asarray(inputs["x2"], np.float32).reshape(B, C, N)
    x1_h = [_pack(x1[b], X1_RANGES) for b in range(B)]
    x2_h = [_pack(x2[b], X2_RANGES) for b in range(B)]
    # scores = (Wq x1)^T (Wk x2) = x1^T (Wq^T Wk) x2: fold both score
    # projections into one by shipping G = Wk^T Wq as the k-side weight;
    # x1 then feeds the score matmuls raw (saves 32 matmuls/core and one
    # rounding on the q side).
    G = (np.asarray(inputs["Wk"], np.float64).T
         @ np.asarray(inputs["Wq"], np.float64)).astype(np.float32)
    wkT = _pack(G, [(0, C)])
    wvT = _pack(np.asarray(inputs["Wv"], np.float32).T, [(0, C)])

    in_maps = [
        {"x1": x1_h[b], "x2": x2_h[b], "wkT": wkT, "wvT": wvT}
        for b in range(B)
    ]
    nc = _get_program()
    res = bass_utils.run_bass_kernel_spmd(nc, in_maps, core_ids=list(range(B)),
                                          trace=TRACE, tmpdir=TRACE_DIR)
    _CACHE["last_results"] = res
    out = np.empty((B, C, N), np.float32)
    for b in range(B):
        out[b] = res.results[b]["outT"].astype(np.float32).T
    return out.reshape(B, C, H, W)


if __name__ == "__main__":
    nc = _build_program()
    n = sum(len(b.instructions) for b in nc.m.functions[0].blocks)
    print(f"program built ok: {n} instructions")
